# revision 2
# baseline (speedup 1.0000x reference)
"""Trainium2 Bass kernel for the DCN cross layer.

Computes out = x0 * (x_cross @ w)[:, None] + b + x_cross for
x0, x_cross: [16384, 4096] f32, w, b: [4096] f32.

Sharding: pure data parallel — batch split across 8 NeuronCores,
w and b replicated. Each core processes a [2048, 4096] shard.

The op is memory-bound (3 HBM streams, no reuse) and the f32 version
sits at the 358 GB/s/core DMA roofline, so all I/O is done in bf16:
the host casts inputs once, the device computes in bf16 with an f32
dot-product accumulator, and the host upcasts the output. Error is
~0.1% in norm, well under the 2e-2 gate.
"""

import sys

import numpy as np

sys.path.insert(0, "/opt/trn_rl_repo")

import ml_dtypes

BF16 = ml_dtypes.bfloat16

N_CORES = 8
BATCH = 16384
D = 4096
ROWS_PER_CORE = BATCH // N_CORES  # 2048
P = 128
RPP = 1  # rows per partition per tile -> DMA transfer size = RPP * 1 MB
BUFS = 4

_NC = None


def _build(rpp=None, bufs=None, tmp_bufs=2, sep_out=False, s_bufs=4):
    """Build + schedule the single-core SPMD program (same on all cores)."""
    from contextlib import ExitStack

    import concourse.tile as tile
    from concourse import bacc, mybir

    rpp = RPP if rpp is None else rpp
    bufs = BUFS if bufs is None else bufs

    bf16 = mybir.dt.bfloat16
    f32 = mybir.dt.float32
    mult = mybir.AluOpType.mult
    add = mybir.AluOpType.add

    nc = bacc.Bacc(
        "TRN2", target_bir_lowering=False, debug=False, num_devices=N_CORES
    )
    x0_d = nc.dram_tensor("x0", [ROWS_PER_CORE, D], bf16, kind="ExternalInput").ap()
    xc_d = nc.dram_tensor(
        "x_cross", [ROWS_PER_CORE, D], bf16, kind="ExternalInput"
    ).ap()
    w_d = nc.dram_tensor("w", [D], bf16, kind="ExternalInput").ap()
    b_d = nc.dram_tensor("b", [D], bf16, kind="ExternalInput").ap()
    out_d = nc.dram_tensor(
        "out", [ROWS_PER_CORE, D], bf16, kind="ExternalOutput"
    ).ap()

    rows_per_tile = P * rpp
    n_tiles = ROWS_PER_CORE // rows_per_tile
    with tile.TileContext(nc) as tc, ExitStack() as ctx:
        consts = ctx.enter_context(tc.tile_pool(name="consts", bufs=1))
        xc_pool = ctx.enter_context(tc.tile_pool(name="xc", bufs=bufs))
        x0_pool = ctx.enter_context(tc.tile_pool(name="x0", bufs=bufs))
        # tmp needs exactly 2 bufs: with 1 the scheduler cannot hoist the
        # next tile's first DVE op ahead of the current tile's last
        # (~45us slower); 3 measured worse than 2
        tmp_pool = ctx.enter_context(tc.tile_pool(name="tmp", bufs=tmp_bufs))
        s_pool = ctx.enter_context(tc.tile_pool(name="s", bufs=s_bufs))
        out_pool = (
            ctx.enter_context(tc.tile_pool(name="outp", bufs=2)) if sep_out else None
        )

        # w and b replicated across all 128 partitions (one-time). The
        # stride-0 DMA broadcast re-reads the same 8 KB per partition but
        # overlaps with the load stream and beat gpsimd.partition_broadcast
        # by ~8 us end-to-end (measured on the f32 version).
        w_t = consts.tile([P, D], bf16)
        b_t = consts.tile([P, D], bf16)
        # issue on the ACT ring (stores come much later there) so the SP
        # ring starts streaming x0/x_cross immediately
        nc.scalar.dma_start(out=w_t[:], in_=w_d.partition_broadcast(P))
        nc.scalar.dma_start(out=b_t[:], in_=b_d.partition_broadcast(P))

        for i in range(n_tiles):
            r0 = i * rows_per_tile
            # [rows_per_tile, D] DRAM block == [P, RPP*D] SBUF tile
            # (partition p holds rows r0 + RPP*p .. r0 + RPP*p + RPP-1)
            xc_t = xc_pool.tile([P, rpp * D], bf16)
            nc.sync.dma_start(
                out=xc_t[:],
                in_=xc_d[r0 : r0 + rows_per_tile, :].rearrange(
                    "(p r) d -> p (r d)", p=P
                ),
            )
            x0_t = x0_pool.tile([P, rpp * D], bf16)
            nc.sync.dma_start(
                out=x0_t[:],
                in_=x0_d[r0 : r0 + rows_per_tile, :].rearrange(
                    "(p r) d -> p (r d)", p=P
                ),
            )

            tmp_t = tmp_pool.tile([P, D], bf16)
            o_t = (
                out_pool.tile([P, rpp * D], bf16, name="o_t", tag="o_t")
                if sep_out
                else xc_t
            )
            s_t = s_pool.tile([P, rpp], f32)
            for j in range(rpp):
                ds = slice(j * D, (j + 1) * D)
                # tmp = xc * w (junk), s = rowsum(xc * w)
                # (tensor_tensor_reduce's native opcode crashes this runtime;
                # scalar_tensor_tensor's accum_out path does the same thing)
                nc.vector.scalar_tensor_tensor(
                    out=tmp_t[:],
                    in0=xc_t[:, ds],
                    scalar=1.0,
                    in1=w_t[:],
                    op0=mult,
                    op1=mult,
                    accum_out=s_t[:, j : j + 1],
                )
                # tmp = x0 * s + xc
                nc.vector.scalar_tensor_tensor(
                    out=tmp_t[:],
                    in0=x0_t[:, ds],
                    scalar=s_t[:, j : j + 1],
                    in1=xc_t[:, ds],
                    op0=mult,
                    op1=add,
                )
                # default: xc slice is dead now; reuse it as the output
                nc.vector.tensor_add(o_t[:, ds], tmp_t[:], b_t[:])
            # store from the ACT HWDGE ring so loads (SP ring) and stores
            # use separate descriptor generators
            nc.scalar.dma_start(
                out=out_d[r0 : r0 + rows_per_tile, :].rearrange(
                    "(p r) d -> p (r d)", p=P
                ),
                in_=o_t[:],
            )

    nc.compile()
    return nc


def _get_nc():
    global _NC
    if _NC is None:
        _NC = _build()
    return _NC


def _run(inputs, trace=False, **spmd_kwargs):
    """Shard, run on 8 cores, gather. Returns (full_output, BassKernelResults)."""
    from concourse.bass_utils import run_bass_kernel_spmd

    nc = _get_nc()

    x0 = np.ascontiguousarray(np.asarray(inputs["x0"]).astype(BF16))
    xc = np.ascontiguousarray(np.asarray(inputs["x_cross"]).astype(BF16))
    w = np.ascontiguousarray(np.asarray(inputs["w"]).astype(BF16))
    b = np.ascontiguousarray(np.asarray(inputs["b"]).astype(BF16))

    in_maps = [
        {
            "x0": x0[i * ROWS_PER_CORE : (i + 1) * ROWS_PER_CORE],
            "x_cross": xc[i * ROWS_PER_CORE : (i + 1) * ROWS_PER_CORE],
            "w": w,
            "b": b,
        }
        for i in range(N_CORES)
    ]

    res = run_bass_kernel_spmd(
        nc, in_maps, core_ids=list(range(N_CORES)), trace=trace, **spmd_kwargs
    )
    out = np.concatenate(
        [res.results[i]["out"] for i in range(N_CORES)], axis=0
    ).astype(np.float32)
    return out, res


def kernel(**inputs: np.ndarray) -> np.ndarray:
    out, _ = _run(inputs)
    return out


# revision 5
# speedup vs baseline: 1.1773x; 1.1773x over previous
"""Trainium2 Bass kernel for the DCN cross layer.

Computes out = x0 * (x_cross @ w)[:, None] + b + x_cross for
x0, x_cross: [16384, 4096] f32, w, b: [4096] f32.

Sharding: pure data parallel — batch split across 8 NeuronCores,
w and b replicated. Each core processes a [2048, 4096] shard.

The op is memory-bound (3 HBM streams, no reuse) and the f32 version
sits at the 358 GB/s/core DMA roofline, so all I/O is done in bf16:
the host casts inputs once, the device computes in bf16 with an f32
dot-product accumulator, and the host upcasts the output. Error is
~0.1% in norm, well under the 2e-2 gate.
"""

import sys

import numpy as np

sys.path.insert(0, "/opt/trn_rl_repo")

import ml_dtypes

BF16 = ml_dtypes.bfloat16

N_CORES = 8
BATCH = 16384
D = 4096
ROWS_PER_CORE = BATCH // N_CORES  # 2048
P = 128
RPP = 1  # rows per partition per tile -> DMA transfer size = RPP * 1 MB
BUFS = 4

_NC = None


def _build(rpp=None, bufs=None, tmp_bufs=3, s_bufs=4, reduce_eng="gpsimd"):
    """Build + schedule the single-core SPMD program (same on all cores).

    Engine split (DVE's scalar_tensor_tensor runs at 1x while tensor_scalar
    hits the 4x DVE mode and tensor_tensor the 2x mode, so the old 3x-stt
    pipeline was vector-bound at ~179us):
      gpsimd: s = rowsum(xc * w)      (stt + accum, the only 1x-rate op)
      DVE:    t = x0 * s              (tensor_scalar, 4x)
              u = t + xc              (tensor_tensor, 2x)
              o = u + b               (tensor_tensor, 2x)
    """
    from contextlib import ExitStack

    import concourse.tile as tile
    from concourse import bacc, mybir

    rpp = RPP if rpp is None else rpp
    bufs = BUFS if bufs is None else bufs

    bf16 = mybir.dt.bfloat16
    f32 = mybir.dt.float32
    mult = mybir.AluOpType.mult
    add = mybir.AluOpType.add

    nc = bacc.Bacc(
        "TRN2", target_bir_lowering=False, debug=False, num_devices=N_CORES
    )
    x0_d = nc.dram_tensor("x0", [ROWS_PER_CORE, D], bf16, kind="ExternalInput").ap()
    xc_d = nc.dram_tensor(
        "x_cross", [ROWS_PER_CORE, D], bf16, kind="ExternalInput"
    ).ap()
    w_d = nc.dram_tensor("w", [D], bf16, kind="ExternalInput").ap()
    b_d = nc.dram_tensor("b", [D], bf16, kind="ExternalInput").ap()
    out_d = nc.dram_tensor(
        "out", [ROWS_PER_CORE, D], bf16, kind="ExternalOutput"
    ).ap()

    rows_per_tile = P * rpp
    n_tiles = ROWS_PER_CORE // rows_per_tile
    with tile.TileContext(nc) as tc, ExitStack() as ctx:
        consts = ctx.enter_context(tc.tile_pool(name="consts", bufs=1))
        xc_pool = ctx.enter_context(tc.tile_pool(name="xc", bufs=bufs))
        x0_pool = ctx.enter_context(tc.tile_pool(name="x0", bufs=bufs))
        junk_pool = ctx.enter_context(tc.tile_pool(name="junk", bufs=2))
        t_pool = ctx.enter_context(tc.tile_pool(name="t", bufs=tmp_bufs))
        u_pool = ctx.enter_context(tc.tile_pool(name="u", bufs=tmp_bufs))
        s_pool = ctx.enter_context(tc.tile_pool(name="s", bufs=s_bufs))

        # w and b replicated across all 128 partitions (one-time). The
        # stride-0 DMA broadcast re-reads the same 8 KB per partition but
        # overlaps with the load stream and beat gpsimd.partition_broadcast
        # by ~8 us end-to-end (measured on the f32 version).
        w_t = consts.tile([P, D], bf16)
        b_t = consts.tile([P, D], bf16)
        # issue on the ACT ring (stores come much later there) so the SP
        # ring starts streaming x0/x_cross immediately
        nc.scalar.dma_start(out=w_t[:], in_=w_d.partition_broadcast(P))
        nc.scalar.dma_start(out=b_t[:], in_=b_d.partition_broadcast(P))

        for i in range(n_tiles):
            r0 = i * rows_per_tile
            # [rows_per_tile, D] DRAM block == [P, RPP*D] SBUF tile
            # (partition p holds rows r0 + RPP*p .. r0 + RPP*p + RPP-1)
            xc_t = xc_pool.tile([P, rpp * D], bf16)
            nc.sync.dma_start(
                out=xc_t[:],
                in_=xc_d[r0 : r0 + rows_per_tile, :].rearrange(
                    "(p r) d -> p (r d)", p=P
                ),
            )
            x0_t = x0_pool.tile([P, rpp * D], bf16)
            nc.sync.dma_start(
                out=x0_t[:],
                in_=x0_d[r0 : r0 + rows_per_tile, :].rearrange(
                    "(p r) d -> p (r d)", p=P
                ),
            )

            junk_t = junk_pool.tile([P, D], bf16)
            s_t = s_pool.tile([P, rpp], f32)
            for j in range(rpp):
                ds = slice(j * D, (j + 1) * D)
                # junk = xc * w (discarded), s = rowsum(xc * w).
                # (TensorScalarPtr fails the Pool-engine ISA check, so the
                # reduce has to stay on the DVE even though it runs at 1x.)
                nc.vector.scalar_tensor_tensor(
                    out=junk_t[:],
                    in0=xc_t[:, ds],
                    scalar=1.0,
                    in1=w_t[:],
                    op0=mult,
                    op1=mult,
                    accum_out=s_t[:, j : j + 1],
                )
                # t = x0 * s on the ACT engine (activation Copy with a
                # per-partition scale AP) to keep the DVE under the DMA
                # roofline budget
                t_t = t_pool.tile([P, D], bf16)
                nc.scalar.mul(t_t[:], x0_t[:, ds], s_t[:, j : j + 1])
                # u = t + xc  (2x mode)
                u_t = u_pool.tile([P, D], bf16)
                nc.vector.tensor_add(u_t[:], t_t[:], xc_t[:, ds])
                # o = u + b   (2x mode); x0 slice is dead, reuse as output
                nc.vector.tensor_add(x0_t[:, ds], u_t[:], b_t[:])
            # store from the ACT HWDGE ring so loads (SP ring) and stores
            # use separate descriptor generators
            nc.scalar.dma_start(
                out=out_d[r0 : r0 + rows_per_tile, :].rearrange(
                    "(p r) d -> p (r d)", p=P
                ),
                in_=x0_t[:],
            )

    nc.compile()
    return nc


def _get_nc():
    global _NC
    if _NC is None:
        _NC = _build()
    return _NC


def _run(inputs, trace=False, **spmd_kwargs):
    """Shard, run on 8 cores, gather. Returns (full_output, BassKernelResults)."""
    from concourse.bass_utils import run_bass_kernel_spmd

    nc = _get_nc()

    x0 = np.ascontiguousarray(np.asarray(inputs["x0"]).astype(BF16))
    xc = np.ascontiguousarray(np.asarray(inputs["x_cross"]).astype(BF16))
    w = np.ascontiguousarray(np.asarray(inputs["w"]).astype(BF16))
    b = np.ascontiguousarray(np.asarray(inputs["b"]).astype(BF16))

    in_maps = [
        {
            "x0": x0[i * ROWS_PER_CORE : (i + 1) * ROWS_PER_CORE],
            "x_cross": xc[i * ROWS_PER_CORE : (i + 1) * ROWS_PER_CORE],
            "w": w,
            "b": b,
        }
        for i in range(N_CORES)
    ]

    res = run_bass_kernel_spmd(
        nc, in_maps, core_ids=list(range(N_CORES)), trace=trace, **spmd_kwargs
    )
    out = np.concatenate(
        [res.results[i]["out"] for i in range(N_CORES)], axis=0
    ).astype(np.float32)
    return out, res


def kernel(**inputs: np.ndarray) -> np.ndarray:
    out, _ = _run(inputs)
    return out
